# revision 5
# baseline (speedup 1.0000x reference)
"""Skip-gram negative-sampling loss on 8 Trainium2 NeuronCores.

Strategy v3 (data-parallel over batch):
  - Each core handles 2048 batch rows and 512 hierarchy pairs.
  - The host shards + lays out the per-core operands exactly as the
    device consumes them (the sharding/packing step): per block b of
    128 rows, a [128, 60*128] bf16 stream holds the out-embedding rows
    for the 60 (pos|neg) context slots of each batch row, and a
    [128, 16, 128] table holds the input-embedding row per batch row.
    All device loads are wide contiguous DMAs (no SWDGE descriptors,
    no Pool-engine time, full 512B+ element bandwidth).
  - Per block: prod = g * broadcast(input_row) on DVE (bf16, 2x mode),
    then a 7-level pairwise-add tree (bf16, 2x mode) reduces D=128 to
    the per-(row,slot) dot. This replaces tensor_reduce (no 2x mode,
    1 elem/cycle) with adds at 2 elem/cycle: the reduce costs ~half
    of the old one and the whole dot pipeline ~2/3.
  - Tail: softplus via max(v,0)+ln(1+exp(-|v|)) with the pos/neg sign
    handled by two tensor_scalar variants on slot ranges; hierarchy
    pairs: sub, square, reduce on a [128, 8, 128] packed tile.
  - Output per core: [128, 2] partial sums, summed on host in f64.
"""

import numpy as np
import ml_dtypes

import concourse.bacc as bacc
import concourse.tile as tile
from concourse import mybir

# Problem shape (hardcoded per contract).
B = 16384
LAST_CAPU = None
LAST_AN = 0
V = 100000
D = 128
C = 10
NEG = 50
PH = 4096
NCORES = 8
P = 128

BL = B // NCORES          # 2048 batch rows per core
HLC = PH // NCORES        # 512 hierarchy pairs per core
NBLK = BL // P            # 16 blocks of 128 rows
SLOTS = C + NEG           # 60 context slots per batch row

BF16 = mybir.dt.bfloat16
F32 = mybir.dt.float32


def make_plan(input_labels, pos_labels, neg_labels, hierarchy_pairs,
              w_in, w_out):
    il = np.asarray(input_labels).astype(np.int64)
    pl = np.asarray(pos_labels).astype(np.int64)
    nl = np.asarray(neg_labels).astype(np.int64)
    hp = np.asarray(hierarchy_pairs).astype(np.int64)

    rlab = np.concatenate([pl, nl], axis=1)  # [B, 60]

    per_core = []
    for k in range(NCORES):
        rows = slice(k * BL, (k + 1) * BL)
        # g stream: [NBLK, 128, SLOTS*D]; partition p of block b holds the
        # 60 context-slot embedding rows of batch row k*BL + 128*b + p.
        gtab = w_out[rlab[rows]].reshape(NBLK, P, SLOTS, D)
        # input rows: [128, NBLK, D] with partition p, slot b = row 128b+p.
        itab = w_in[il[rows]].reshape(NBLK, P, D).transpose(1, 0, 2).copy()
        # hierarchy: [128, 8, D]; pair i at partition i%128: left in slot
        # i//128 (0..3), right in slot 4 + i//128.
        hl = w_in[hp[k * HLC:(k + 1) * HLC, 0]]
        hr = w_in[hp[k * HLC:(k + 1) * HLC, 1]]
        htab = np.concatenate([
            hl.reshape(HLC // P, P, D).transpose(1, 0, 2),
            hr.reshape(HLC // P, P, D).transpose(1, 0, 2),
        ], axis=1).copy()
        per_core.append({
            "gt": np.ascontiguousarray(gtab),
            "itab": np.ascontiguousarray(itab),
            "htab": np.ascontiguousarray(htab),
        })
    return per_core


def build_program(capu=None, enable_asserts=False, repeat=1, an=0,
                  gbufs=3, sbufs=2):
    nc = bacc.Bacc(
        "TRN2",
        target_bir_lowering=False,
        debug=False,
        enable_asserts=enable_asserts,
        num_devices=NCORES,
    )

    gt_d = nc.dram_tensor("gt", [NBLK, P, SLOTS, D], BF16,
                          kind="ExternalInput").ap()
    itab_d = nc.dram_tensor("itab", [P, NBLK, D], BF16,
                            kind="ExternalInput").ap()
    htab_d = nc.dram_tensor("htab", [P, 2 * HLC // P, D], BF16,
                            kind="ExternalInput").ap()
    out_d = nc.dram_tensor("out", [P, 2], F32, kind="ExternalOutput").ap()

    with tile.TileContext(nc) as tc:
        with (
            tc.tile_pool(name="inp", bufs=2) as inpp,
            tc.tile_pool(name="gath", bufs=gbufs) as gp,
            tc.tile_pool(name="prod", bufs=sbufs) as prodp,
            tc.tile_pool(name="dots", bufs=2) as dotsp,
            tc.tile_pool(name="end", bufs=2) as endp,
        ):
          for _rep in range(repeat):
            itile = inpp.tile([P, NBLK, D], BF16, tag="itile")
            htile = inpp.tile([P, 2 * HLC // P, D], BF16, tag="htile")
            nc.sync.dma_start(itile[:], itab_d)
            nc.sync.dma_start(htile[:], htab_d)

            dots = dotsp.tile([P, NBLK, SLOTS], F32, tag="dots")

            for b in range(NBLK):
                g = gp.tile([P, SLOTS, D], BF16, tag="g")
                nc.sync.dma_start(g[:], gt_d[b])
                prod = prodp.tile([P, SLOTS, D], BF16, tag="prod")
                nc.vector.tensor_tensor(
                    out=prod[:],
                    in0=g[:],
                    in1=itile[:, b : b + 1, :].broadcast_to([P, SLOTS, D]),
                    op=mybir.AluOpType.mult,
                )
                # pairwise-add tree over D: 128 -> 1, bf16 until the last
                # level (f32 out). Ping-pong between two halves of scratch.
                ta = prodp.tile([P, SLOTS, 64], BF16, tag="ta")
                tb = prodp.tile([P, SLOTS, 32], BF16, tag="tb")
                nc.vector.tensor_tensor(
                    out=ta[:], in0=prod[:, :, 0:64], in1=prod[:, :, 64:128],
                    op=mybir.AluOpType.add,
                )
                nc.vector.tensor_tensor(
                    out=tb[:], in0=ta[:, :, 0:32], in1=ta[:, :, 32:64],
                    op=mybir.AluOpType.add,
                )
                nc.vector.tensor_tensor(
                    out=ta[:, :, 0:16], in0=tb[:, :, 0:16], in1=tb[:, :, 16:32],
                    op=mybir.AluOpType.add,
                )
                nc.vector.tensor_tensor(
                    out=tb[:, :, 0:8], in0=ta[:, :, 0:8], in1=ta[:, :, 8:16],
                    op=mybir.AluOpType.add,
                )
                nc.vector.tensor_tensor(
                    out=ta[:, :, 0:4], in0=tb[:, :, 0:4], in1=tb[:, :, 4:8],
                    op=mybir.AluOpType.add,
                )
                nc.vector.tensor_tensor(
                    out=tb[:, :, 0:2], in0=ta[:, :, 0:2], in1=ta[:, :, 2:4],
                    op=mybir.AluOpType.add,
                )
                nc.vector.tensor_tensor(
                    out=dots[:, b, :], in0=tb[:, :, 0:1], in1=tb[:, :, 1:2],
                    op=mybir.AluOpType.add,
                )

            # hierarchy: htile slots 0:4 = left rows, 4:8 = right rows
            nh = HLC // P  # 4
            dif = endp.tile([P, nh, D], BF16, tag="dif")
            nc.vector.tensor_tensor(
                out=dif[:], in0=htile[:, 0:nh, :], in1=htile[:, nh : 2 * nh, :],
                op=mybir.AluOpType.subtract,
            )
            sq = endp.tile([P, nh, D], F32, tag="sq")
            nc.scalar.activation(
                out=sq[:], in_=dif[:],
                func=mybir.ActivationFunctionType.Square,
            )
            h_acc = endp.tile([P, 1], F32, tag="h_acc")
            nc.vector.reduce_sum(out=h_acc[:], in_=sq[:], axis=mybir.AxisListType.XY)

            # softplus(v) = max(v,0) + ln(1+exp(-|v|));
            # v = -dot for pos slots (j<10), +dot for neg slots.
            eb = endp.tile([P, 2, NBLK, SLOTS], F32, tag="eb")
            nc.vector.tensor_scalar(
                out=eb[:, 0, :, 0:C], in0=dots[:, :, 0:C],
                scalar1=0.0, scalar2=-1.0,
                op0=mybir.AluOpType.min, op1=mybir.AluOpType.mult,
            )
            nc.vector.tensor_scalar(
                out=eb[:, 0, :, C:SLOTS], in0=dots[:, :, C:SLOTS],
                scalar1=0.0, scalar2=None, op0=mybir.AluOpType.max,
            )
            absv = endp.tile([P, NBLK, SLOTS], F32, tag="absv")
            nc.scalar.activation(
                out=absv[:], in_=dots[:],
                func=mybir.ActivationFunctionType.Abs,
            )
            expv = endp.tile([P, NBLK, SLOTS], F32, tag="expv")
            nc.scalar.activation(
                out=expv[:], in_=absv[:],
                func=mybir.ActivationFunctionType.Exp, scale=-1.0,
            )
            nc.scalar.activation(
                out=eb[:, 1, :, :], in_=expv[:],
                func=mybir.ActivationFunctionType.Ln, bias=1.0,
            )
            r1 = endp.tile([P, 2, NBLK], F32, tag="r1")
            nc.vector.reduce_sum(out=r1[:], in_=eb[:], axis=mybir.AxisListType.X)
            s_acc = endp.tile([P, 1], F32, tag="s_acc")
            nc.vector.reduce_sum(out=s_acc[:], in_=r1[:], axis=mybir.AxisListType.XY)

            out_sb = endp.tile([P, 2], F32, tag="out_sb")
            nc.vector.tensor_copy(out_sb[:, 0:1], s_acc[:])
            nc.vector.tensor_copy(out_sb[:, 1:2], h_acc[:])
            nc.sync.dma_start(out_d, out_sb[:])

    nc.compile()
    return nc


def prepare(input_labels, pos_labels, neg_labels, hierarchy_pairs,
            in_embed_w, out_embed_w):
    w_in = np.asarray(in_embed_w, dtype=np.float32).astype(ml_dtypes.bfloat16)
    w_out = np.asarray(out_embed_w, dtype=np.float32).astype(ml_dtypes.bfloat16)

    per_core = make_plan(input_labels, pos_labels, neg_labels,
                         hierarchy_pairs, w_in, w_out)
    nc = build_program()
    return nc, per_core, None


def combine_results(per_core_outs, pads):
    s_total = 0.0
    h_total = 0.0
    for r in per_core_outs:
        o = r["out"].astype(np.float64)
        s_total += o[:, 0].sum()
        h_total += o[:, 1].sum()
    loss_graph = s_total / B
    loss_h = 0.5 * 1e-8 * h_total
    return (np.float32(loss_graph + loss_h), np.float32(loss_h))


def run_on_hw(nc, in_maps, **kwargs):
    from concourse.bass_utils import run_bass_kernel_spmd

    return run_bass_kernel_spmd(
        nc, in_maps, core_ids=list(range(NCORES)), **kwargs
    )


def kernel(input_labels, pos_labels, neg_labels, hierarchy_pairs,
           in_embed_w, out_embed_w):
    nc, in_maps, pads = prepare(
        input_labels, pos_labels, neg_labels, hierarchy_pairs,
        in_embed_w, out_embed_w,
    )
    res = run_on_hw(nc, in_maps)
    return combine_results(res.results, pads)
